# revision 26
# baseline (speedup 1.0000x reference)
"""Int8SymmetricLinear Trainium2 kernel.

Computes out = x @ (weight.astype(f32) * weight_scale).T + bias
  x: [4, 2048, 4096] f32, weight: [11008, 4096] int8,
  weight_scale: [11008, 1] f32, bias: [11008] f32
  out: [4, 2048, 11008] f32

Strategy: token-parallel across 8 NeuronCores (1024 tokens each, full
weight replicated). Per core, x^T stays SBUF-resident as fp16 (single
pass; int8 weights are exact in fp16, measured rel err ~2e-4 vs the
2e-2 gate); int8 weights stream per 128-row out-feature tile as fp16.
PE computes out^T[o, t] tiles = w_tile.T @ x_tile with accumulating
matmuls over 32 k-tiles. DVE applies per-partition scale+bias fused.
Host packs/unpacks layouts (transposes are free off-device).

Measured on trn2 (core 0 profile): ~1.216 ms vs a 1.188 ms PE-stream
floor (5504 MMs x 215.8 ns, LDWEIGHTS fully hidden, zero steady-state
PE gaps; every warm MM gap is exactly 215-216 ns). Startup: weight
row 0 is one DMA ahead of the x-resident load (first matmul ~12.5 us,
gated by the ~10 us NEFF preamble + 1.25 MB at ~380 GB/s); row 1 is 4
k-chunks behind x so ot=1 starts when its first chunk lands after x
drains. Residual overhead: ~2.6 us HAM cold-clock ramp (pre-warm
tried 4 ways, always lost — the gate re-throttles across a DMA-bound
startup), ~2.3 us BW-bound x-gating idle, ~6 us tail (last-tile
dequant split ACT||DVE, then teardown barriers). A dual bf16 hi/lo
mode ("bf16x2", rel err 2.5e-6) is kept for reference; 2x slower.
"""

import sys

sys.path.insert(0, "/opt/trn_rl_repo")

import ml_dtypes
import numpy as np

BF16 = ml_dtypes.bfloat16

# Full-problem constants (hardcoded per contract)
B, S, IN, OUT = 4, 2048, 4096, 11008
N_CORES = 8
P = 128

_NC_CACHE = {}


def _build_nc(n_kt, n_ot, t_core, t_free, mode="bf16x2", reps=1, wbufs=3, obufs=4, psbufs=4):
    """Build the per-core Bass program (same program on all 8 cores).

    mode: "bf16x2" = dual-pass hi/lo bf16 (near-fp32 accuracy)
          "fp16"   = single-pass fp16 (~1.5e-4 absmax-rel)
    reps: >1 wraps the compute body in a hardware loop (timing only).
    """
    import concourse.bass as bass
    import concourse.mybir as mybir
    import concourse.tile as tile
    from concourse import bacc
    from contextlib import ExitStack

    f32 = mybir.dt.float32
    xdt = mybir.dt.bfloat16 if mode == "bf16x2" else mybir.dt.float16
    n_th = t_core // t_free
    dual = mode == "bf16x2"

    nc = bacc.Bacc("TRN2", target_bir_lowering=False, debug=False)

    x_names = ["x_hi", "x_lo"] if dual else ["x_hi"]
    x_d = {
        nm: nc.dram_tensor(nm, [n_kt, P, t_core], xdt, kind="ExternalInput").ap()
        for nm in x_names
    }
    w_d = nc.dram_tensor("w", [n_ot, P, n_kt, P], xdt, kind="ExternalInput").ap()
    sc_d = nc.dram_tensor("scale", [P, n_ot], f32, kind="ExternalInput").ap()
    bi_d = nc.dram_tensor("bias", [P, n_ot], f32, kind="ExternalInput").ap()
    out_d = nc.dram_tensor("out", [n_ot * P, t_core], f32, kind="ExternalOutput").ap()

    with tile.TileContext(nc) as tc:
        n_wc = 4  # k-chunks for the first two weight rows
        kc = n_kt // n_wc
        with (
            tc.tile_pool(name="xpool", bufs=1) as xpool,
            tc.tile_pool(name="wpool", bufs=wbufs) as wpool,
            tc.tile_pool(name="w01pool", bufs=1) as w01pool,
            tc.tile_pool(name="cpool", bufs=1) as cpool,
            tc.tile_pool(name="opool", bufs=obufs) as opool,
            tc.tile_pool(name="pspool", bufs=psbufs, space="PSUM") as pspool,
        ):
            # Weight row 0 heads the DMA stream as ONE transfer (chunking it
            # lets the engines interleave the chunks' packets and the first
            # chunk finishes LATER than the whole un-chunked row — measured
            # +3us on the first matmul). Row 1 (issued below, behind x) IS
            # chunked: there the win is that ot=1 starts as soon as its
            # first chunk lands after the x-resident load drains, instead
            # of stalling ~2us for the whole row.
            w0 = wpool.tile([P, n_kt, P], xdt)
            nc.sync.dma_start(out=w0[:], in_=w_d[0])
            # Row 1 rides interleaved WITH the x stream (as 4 small chunks,
            # each issued just ahead of the x k-range that consumes it):
            # ot=0 and ot=1 are computed in ONE interleaved k-loop below,
            # which halves the PE's x-consumption rate (~600 -> ~300 GB/s)
            # to below the ~390 GB/s DMA delivery rate, eliminating the
            # x-gated idle entirely without delaying x tile 0.
            w1c = [
                w01pool.tile([P, kc, P], xdt, tag=f"w1c{j}", name=f"w1c{j}")
                for j in range(n_wc)
            ]
            x_sb = {nm: [] for nm in x_names}
            for i in range(n_kt):
                if i >= 1 and (i - 1) % kc == 0:
                    j = (i - 1) // kc
                    nc.sync.dma_start(
                        out=w1c[j][:], in_=w_d[1][:, j * kc : (j + 1) * kc, :]
                    )
                for nm in x_names:
                    t = xpool.tile([P, t_core], xdt, tag=f"{nm}_{i}")
                    nc.sync.dma_start(out=t[:], in_=x_d[nm][i])
                    x_sb[nm].append(t)
            sc = cpool.tile([P, n_ot], f32)
            bi = cpool.tile([P, n_ot], f32)
            nc.sync.dma_start(out=sc[:], in_=sc_d[:])
            nc.sync.dma_start(out=bi[:], in_=bi_d[:])

            # NOTE: PE pre-warm (junk matmuls during the DMA wait, to defeat
            # the ~3us HAM cold-clock ramp) was tried 4 ways and always lost:
            # the warm->idle->x-gated-stream pattern makes the clock gate
            # re-throttle mid-stream, which costs more than the ramp.

            tsls = [bass.ds(th * t_free, t_free) for th in range(n_th)]

            def post(ot, pss, last=False):
                for th in range(n_th):
                    osb = opool.tile([P, t_free], f32, tag="osb", name="osb")
                    if last and th == 0:
                        # Tail: the last tile's two post-ops are the only
                        # ones on the critical path. Put th0 on the Scalar
                        # engine so both halves dequant in parallel
                        # (different PSUM banks, so ACT+DVE may overlap).
                        nc.scalar.activation(
                            out=osb[:],
                            in_=pss[th][:],
                            func=mybir.ActivationFunctionType.Identity,
                            scale=sc[:, ot : ot + 1],
                            bias=bi[:, ot : ot + 1],
                        )
                    else:
                        nc.vector.tensor_scalar(
                            out=osb[:],
                            in0=pss[th][:],
                            scalar1=sc[:, ot : ot + 1],
                            scalar2=bi[:, ot : ot + 1],
                            op0=mybir.AluOpType.mult,
                            op1=mybir.AluOpType.add,
                        )
                    nc.sync.dma_start(
                        out=out_d[ot * P : (ot + 1) * P, tsls[th]], in_=osb[:]
                    )

            interleave01 = reps == 1 and not dual and n_ot >= 2

            def body(_rep=None):
                start_ot = 0
                if interleave01:
                    # ot=0 and ot=1 in one interleaved k-loop (4 MMs per x
                    # k-tile instead of 2) so the PE never outruns the
                    # startup DMA stream. Uses 4 PSUM banks (pool = 4 bufs).
                    start_ot = 2
                    pss01 = [
                        [
                            pspool.tile(
                                [P, t_free], f32, tag=f"ps{th}", name=f"ps{th}"
                            )
                            for th in range(n_th)
                        ]
                        for _g in (0, 1)
                    ]
                    for i in range(n_kt):
                        for g in (0, 1):
                            wk = (
                                w0[:, i, :]
                                if g == 0
                                else w1c[i // kc][:, i % kc, :]
                            )
                            for th in range(n_th):
                                nc.tensor.matmul(
                                    pss01[g][th][:],
                                    wk,
                                    x_sb["x_hi"][i][:, tsls[th]],
                                    start=(i == 0),
                                    stop=(i == n_kt - 1),
                                )
                    post(0, pss01[0])
                    post(1, pss01[1])
                for ot in range(start_ot, n_ot):
                    w = wpool.tile([P, n_kt, P], xdt)
                    nc.sync.dma_start(out=w[:], in_=w_d[ot])
                    # Interleave all t-halves inside the k-loop: one weight
                    # tile (LDWEIGHTS) feeds n_th * passes matmuls.
                    pss = [
                        pspool.tile([P, t_free], f32, tag=f"ps{th}", name=f"ps{th}")
                        for th in range(n_th)
                    ]
                    for i in range(n_kt):
                        wk = w[:, i, :]
                        for th in range(n_th):
                            nc.tensor.matmul(
                                pss[th][:],
                                wk,
                                x_sb["x_hi"][i][:, tsls[th]],
                                start=(i == 0),
                                stop=(not dual and i == n_kt - 1),
                            )
                            if dual:
                                nc.tensor.matmul(
                                    pss[th][:],
                                    wk,
                                    x_sb["x_lo"][i][:, tsls[th]],
                                    start=False,
                                    stop=(i == n_kt - 1),
                                )
                    post(ot, pss, last=(ot == n_ot - 1))

            if reps > 1:
                with tc.For_i(0, reps, 1):
                    body()
            else:
                body()

    nc.compile()
    return nc


def _get_nc(n_kt, n_ot, t_core, t_free, mode="bf16x2", reps=1, **kw):
    key = (n_kt, n_ot, t_core, t_free, mode, reps, tuple(sorted(kw.items())))
    if key not in _NC_CACHE:
        _NC_CACHE[key] = _build_nc(n_kt, n_ot, t_core, t_free, mode, reps, **kw)
    return _NC_CACHE[key]


def _pack_x(x2, t0, t1, mode):
    """x2 [T, K] f32 -> dict of [K/128, 128, t1-t0] device tensors."""
    xs = x2[t0:t1]
    n_kt = xs.shape[1] // P

    def pack(a):
        # [t, K] -> [n_kt, P, t]
        return np.ascontiguousarray(a.reshape(t1 - t0, n_kt, P).transpose(1, 2, 0))

    if mode == "bf16x2":
        hi = xs.astype(BF16)
        lo = (xs - hi.astype(np.float32)).astype(BF16)
        return {"x_hi": pack(hi), "x_lo": pack(lo)}
    else:
        return {"x_hi": pack(xs.astype(np.float16))}


def prep_inputs(x2, weight, weight_scale, bias, mode="bf16x2"):
    T, K = x2.shape
    O = weight.shape[0]
    t_core = T // N_CORES
    n_kt = K // P
    n_ot = O // P
    npdt = BF16 if mode == "bf16x2" else np.float16

    w_pack = np.ascontiguousarray(
        weight.reshape(n_ot, P, n_kt, P).transpose(0, 3, 2, 1).astype(npdt)
    )
    sc_pack = np.ascontiguousarray(weight_scale.reshape(n_ot, P).T.astype(np.float32))
    bi_pack = np.ascontiguousarray(bias.reshape(n_ot, P).T.astype(np.float32))

    in_maps = []
    for c in range(N_CORES):
        m = _pack_x(x2, c * t_core, (c + 1) * t_core, mode)
        m.update({"w": w_pack, "scale": sc_pack, "bias": bi_pack})
        in_maps.append(m)
    return in_maps


def gather_out(results, T, O):
    out = np.empty((T, O), dtype=np.float32)
    t_core = T // N_CORES
    for c in range(N_CORES):
        out[c * t_core : (c + 1) * t_core] = results[c]["out"].T
    return out


def run_sharded(x2, weight, weight_scale, bias, trace=False, mode="fp16"):
    """x2: [T, K] f32 (flattened tokens). Returns ([T, O] f32, BassKernelResults)."""
    from concourse.bass_utils import run_bass_kernel_spmd

    T, K = x2.shape
    O = weight.shape[0]
    t_core = T // N_CORES
    nc = _get_nc(K // P, O // P, t_core, min(512, t_core), mode)
    in_maps = prep_inputs(x2, weight, weight_scale, bias, mode)
    res = run_bass_kernel_spmd(nc, in_maps, list(range(N_CORES)), trace=trace)
    return gather_out(res.results, T, O), res


def kernel(x, weight, weight_scale, bias):
    x = np.asarray(x, dtype=np.float32)
    weight = np.asarray(weight)
    weight_scale = np.asarray(weight_scale, dtype=np.float32)
    bias = np.asarray(bias, dtype=np.float32)

    x2 = x.reshape(B * S, IN)
    out, _ = run_sharded(x2, weight, weight_scale, bias, trace=False)
    return out.reshape(B, S, OUT)



# revision 28
# speedup vs baseline: 1.0048x; 1.0048x over previous
"""Int8SymmetricLinear Trainium2 kernel.

Computes out = x @ (weight.astype(f32) * weight_scale).T + bias
  x: [4, 2048, 4096] f32, weight: [11008, 4096] int8,
  weight_scale: [11008, 1] f32, bias: [11008] f32
  out: [4, 2048, 11008] f32

Strategy: token-parallel across 8 NeuronCores (1024 tokens each, full
weight replicated). Per core, x^T stays SBUF-resident as fp16 (single
pass; int8 weights are exact in fp16, measured rel err ~2e-4 vs the
2e-2 gate); int8 weights stream per 128-row out-feature tile as fp16.
PE computes out^T[o, t] tiles = w_tile.T @ x_tile with accumulating
matmuls over 32 k-tiles. DVE applies per-partition scale+bias fused.
Host packs/unpacks layouts (transposes are free off-device).

Measured on trn2 (core 0 profile): ~1.213 ms vs a 1.188 ms PE-stream
floor (5504 MMs x 215.8 ns, LDWEIGHTS fully hidden; every warm MM gap
is exactly 215-216 ns and total PE idle is ZERO). Startup: weight
rows 0 and 1 (row 1 as 4 k-chunks) are DMA'd ahead of the x-resident
load, and ot=0/ot=1 are computed in ONE interleaved k-loop — 4 MMs
per x k-tile halves the PE's x-consumption rate to ~300 GB/s, under
the ~390 GB/s DMA delivery rate, so the x-gated phase has no stalls.
Residual overhead: ~16 us to first MM (~10 us NEFF preamble + 2.25 MB
w0+w1 ahead of x0), ~2.5 us HAM cold-clock ramp (pre-warm tried 4
ways, always lost — the gate re-throttles across a DMA-bound
startup), ~6 us tail (last-tile dequant split ACT||DVE, then teardown
barriers). A dual bf16 hi/lo mode ("bf16x2", rel err 2.5e-6) is kept
for reference; 2x slower.
"""

import sys

sys.path.insert(0, "/opt/trn_rl_repo")

import ml_dtypes
import numpy as np

BF16 = ml_dtypes.bfloat16

# Full-problem constants (hardcoded per contract)
B, S, IN, OUT = 4, 2048, 4096, 11008
N_CORES = 8
P = 128

_NC_CACHE = {}


def _build_nc(n_kt, n_ot, t_core, t_free, mode="bf16x2", reps=1, wbufs=3, obufs=4, psbufs=4):
    """Build the per-core Bass program (same program on all 8 cores).

    mode: "bf16x2" = dual-pass hi/lo bf16 (near-fp32 accuracy)
          "fp16"   = single-pass fp16 (~1.5e-4 absmax-rel)
    reps: >1 wraps the compute body in a hardware loop (timing only).
    """
    import concourse.bass as bass
    import concourse.mybir as mybir
    import concourse.tile as tile
    from concourse import bacc
    from contextlib import ExitStack

    f32 = mybir.dt.float32
    xdt = mybir.dt.bfloat16 if mode == "bf16x2" else mybir.dt.float16
    n_th = t_core // t_free
    dual = mode == "bf16x2"

    nc = bacc.Bacc("TRN2", target_bir_lowering=False, debug=False)

    x_names = ["x_hi", "x_lo"] if dual else ["x_hi"]
    x_d = {
        nm: nc.dram_tensor(nm, [n_kt, P, t_core], xdt, kind="ExternalInput").ap()
        for nm in x_names
    }
    w_d = nc.dram_tensor("w", [n_ot, P, n_kt, P], xdt, kind="ExternalInput").ap()
    sc_d = nc.dram_tensor("scale", [P, n_ot], f32, kind="ExternalInput").ap()
    bi_d = nc.dram_tensor("bias", [P, n_ot], f32, kind="ExternalInput").ap()
    out_d = nc.dram_tensor("out", [n_ot * P, t_core], f32, kind="ExternalOutput").ap()

    with tile.TileContext(nc) as tc:
        n_wc = 4  # k-chunks for the first two weight rows
        kc = n_kt // n_wc
        with (
            tc.tile_pool(name="xpool", bufs=1) as xpool,
            tc.tile_pool(name="wpool", bufs=wbufs) as wpool,
            tc.tile_pool(name="w01pool", bufs=1) as w01pool,
            tc.tile_pool(name="cpool", bufs=1) as cpool,
            tc.tile_pool(name="opool", bufs=obufs) as opool,
            tc.tile_pool(name="pspool", bufs=psbufs, space="PSUM") as pspool,
        ):
            # Weight row 0 heads the DMA stream as ONE transfer (chunking it
            # lets the engines interleave the chunks' packets and the first
            # chunk finishes LATER than the whole un-chunked row — measured
            # +3us on the first matmul). Row 1 (issued below, behind x) IS
            # chunked: there the win is that ot=1 starts as soon as its
            # first chunk lands after the x-resident load drains, instead
            # of stalling ~2us for the whole row.
            w0 = wpool.tile([P, n_kt, P], xdt)
            nc.sync.dma_start(out=w0[:], in_=w_d[0])
            # Row 1 rides ahead of x too (as 4 small chunks): ot=0 and ot=1
            # are computed in ONE interleaved k-loop below, which halves the
            # PE's x-consumption rate (~600 -> ~300 GB/s) to below the ~390
            # GB/s DMA delivery rate, eliminating the x-gated idle entirely
            # (measured: zero PE gaps over the whole run). Spreading these
            # chunks into the x stream to start the first MM sooner was
            # tried and lost: the delivery margin (~14%) is too thin and
            # the early stream stalls repeatedly.
            w1c = [
                w01pool.tile([P, kc, P], xdt, tag=f"w1c{j}", name=f"w1c{j}")
                for j in range(n_wc)
            ]
            for j in range(n_wc):
                nc.sync.dma_start(
                    out=w1c[j][:], in_=w_d[1][:, j * kc : (j + 1) * kc, :]
                )
            x_sb = {nm: [] for nm in x_names}
            for i in range(n_kt):
                for nm in x_names:
                    t = xpool.tile([P, t_core], xdt, tag=f"{nm}_{i}")
                    nc.sync.dma_start(out=t[:], in_=x_d[nm][i])
                    x_sb[nm].append(t)
            sc = cpool.tile([P, n_ot], f32)
            bi = cpool.tile([P, n_ot], f32)
            nc.sync.dma_start(out=sc[:], in_=sc_d[:])
            nc.sync.dma_start(out=bi[:], in_=bi_d[:])

            # NOTE: PE pre-warm (junk matmuls during the DMA wait, to defeat
            # the ~3us HAM cold-clock ramp) was tried 4 ways and always lost:
            # the warm->idle->x-gated-stream pattern makes the clock gate
            # re-throttle mid-stream, which costs more than the ramp.

            tsls = [bass.ds(th * t_free, t_free) for th in range(n_th)]

            def post(ot, pss, last=False):
                for th in range(n_th):
                    osb = opool.tile([P, t_free], f32, tag="osb", name="osb")
                    if last and th == 0:
                        # Tail: the last tile's two post-ops are the only
                        # ones on the critical path. Put th0 on the Scalar
                        # engine so both halves dequant in parallel
                        # (different PSUM banks, so ACT+DVE may overlap).
                        nc.scalar.activation(
                            out=osb[:],
                            in_=pss[th][:],
                            func=mybir.ActivationFunctionType.Identity,
                            scale=sc[:, ot : ot + 1],
                            bias=bi[:, ot : ot + 1],
                        )
                    else:
                        nc.vector.tensor_scalar(
                            out=osb[:],
                            in0=pss[th][:],
                            scalar1=sc[:, ot : ot + 1],
                            scalar2=bi[:, ot : ot + 1],
                            op0=mybir.AluOpType.mult,
                            op1=mybir.AluOpType.add,
                        )
                    nc.sync.dma_start(
                        out=out_d[ot * P : (ot + 1) * P, tsls[th]], in_=osb[:]
                    )

            interleave01 = reps == 1 and not dual and n_ot >= 2

            def body(_rep=None):
                start_ot = 0
                if interleave01:
                    # ot=0 and ot=1 in one interleaved k-loop (4 MMs per x
                    # k-tile instead of 2) so the PE never outruns the
                    # startup DMA stream. Uses 4 PSUM banks (pool = 4 bufs).
                    start_ot = 2
                    pss01 = [
                        [
                            pspool.tile(
                                [P, t_free], f32, tag=f"ps{th}", name=f"ps{th}"
                            )
                            for th in range(n_th)
                        ]
                        for _g in (0, 1)
                    ]
                    for i in range(n_kt):
                        for g in (0, 1):
                            wk = (
                                w0[:, i, :]
                                if g == 0
                                else w1c[i // kc][:, i % kc, :]
                            )
                            for th in range(n_th):
                                nc.tensor.matmul(
                                    pss01[g][th][:],
                                    wk,
                                    x_sb["x_hi"][i][:, tsls[th]],
                                    start=(i == 0),
                                    stop=(i == n_kt - 1),
                                )
                    post(0, pss01[0])
                    post(1, pss01[1])
                for ot in range(start_ot, n_ot):
                    w = wpool.tile([P, n_kt, P], xdt)
                    nc.sync.dma_start(out=w[:], in_=w_d[ot])
                    # Interleave all t-halves inside the k-loop: one weight
                    # tile (LDWEIGHTS) feeds n_th * passes matmuls.
                    pss = [
                        pspool.tile([P, t_free], f32, tag=f"ps{th}", name=f"ps{th}")
                        for th in range(n_th)
                    ]
                    for i in range(n_kt):
                        wk = w[:, i, :]
                        for th in range(n_th):
                            nc.tensor.matmul(
                                pss[th][:],
                                wk,
                                x_sb["x_hi"][i][:, tsls[th]],
                                start=(i == 0),
                                stop=(not dual and i == n_kt - 1),
                            )
                            if dual:
                                nc.tensor.matmul(
                                    pss[th][:],
                                    wk,
                                    x_sb["x_lo"][i][:, tsls[th]],
                                    start=False,
                                    stop=(i == n_kt - 1),
                                )
                    post(ot, pss, last=(ot == n_ot - 1))

            if reps > 1:
                with tc.For_i(0, reps, 1):
                    body()
            else:
                body()

    nc.compile()
    return nc


def _get_nc(n_kt, n_ot, t_core, t_free, mode="bf16x2", reps=1, **kw):
    key = (n_kt, n_ot, t_core, t_free, mode, reps, tuple(sorted(kw.items())))
    if key not in _NC_CACHE:
        _NC_CACHE[key] = _build_nc(n_kt, n_ot, t_core, t_free, mode, reps, **kw)
    return _NC_CACHE[key]


def _pack_x(x2, t0, t1, mode):
    """x2 [T, K] f32 -> dict of [K/128, 128, t1-t0] device tensors."""
    xs = x2[t0:t1]
    n_kt = xs.shape[1] // P

    def pack(a):
        # [t, K] -> [n_kt, P, t]
        return np.ascontiguousarray(a.reshape(t1 - t0, n_kt, P).transpose(1, 2, 0))

    if mode == "bf16x2":
        hi = xs.astype(BF16)
        lo = (xs - hi.astype(np.float32)).astype(BF16)
        return {"x_hi": pack(hi), "x_lo": pack(lo)}
    else:
        return {"x_hi": pack(xs.astype(np.float16))}


def prep_inputs(x2, weight, weight_scale, bias, mode="bf16x2"):
    T, K = x2.shape
    O = weight.shape[0]
    t_core = T // N_CORES
    n_kt = K // P
    n_ot = O // P
    npdt = BF16 if mode == "bf16x2" else np.float16

    w_pack = np.ascontiguousarray(
        weight.reshape(n_ot, P, n_kt, P).transpose(0, 3, 2, 1).astype(npdt)
    )
    sc_pack = np.ascontiguousarray(weight_scale.reshape(n_ot, P).T.astype(np.float32))
    bi_pack = np.ascontiguousarray(bias.reshape(n_ot, P).T.astype(np.float32))

    in_maps = []
    for c in range(N_CORES):
        m = _pack_x(x2, c * t_core, (c + 1) * t_core, mode)
        m.update({"w": w_pack, "scale": sc_pack, "bias": bi_pack})
        in_maps.append(m)
    return in_maps


def gather_out(results, T, O):
    out = np.empty((T, O), dtype=np.float32)
    t_core = T // N_CORES
    for c in range(N_CORES):
        out[c * t_core : (c + 1) * t_core] = results[c]["out"].T
    return out


def run_sharded(x2, weight, weight_scale, bias, trace=False, mode="fp16"):
    """x2: [T, K] f32 (flattened tokens). Returns ([T, O] f32, BassKernelResults)."""
    from concourse.bass_utils import run_bass_kernel_spmd

    T, K = x2.shape
    O = weight.shape[0]
    t_core = T // N_CORES
    nc = _get_nc(K // P, O // P, t_core, min(512, t_core), mode)
    in_maps = prep_inputs(x2, weight, weight_scale, bias, mode)
    res = run_bass_kernel_spmd(nc, in_maps, list(range(N_CORES)), trace=trace)
    return gather_out(res.results, T, O), res


def kernel(x, weight, weight_scale, bias):
    x = np.asarray(x, dtype=np.float32)
    weight = np.asarray(weight)
    weight_scale = np.asarray(weight_scale, dtype=np.float32)
    bias = np.asarray(bias, dtype=np.float32)

    x2 = x.reshape(B * S, IN)
    out, _ = run_sharded(x2, weight, weight_scale, bias, trace=False)
    return out.reshape(B, S, OUT)

